# revision 41
# baseline (speedup 1.0000x reference)
"""Trainium2 Bass kernel for nn_LoRAExpert (moe_routing).

Per token t (expert e_t from contiguous group_sizes, adapter a_t):

    out[t] = x[t] @ W[e_t] + s_{a_t} * (x[t] @ A[a_t, e_t]) @ B[a_t, e_t]

Strategy (expert-parallel over 8 NeuronCores):
  - Host routes tokens: x is already expert-sorted, so core e gets the
    contiguous slice x[off_e : off_e + gs_e], padded to a common `cap`.
  - LoRA routing trick: with A=8 adapters and rank R=16, the per-expert
    concatenation A_cat = [A[0,e] .. A[7,e]] is [1024, 128]. Compute
    inter_all = x @ A_cat densely for ALL adapters, then multiply by a
    per-token mask M[j, t] = s_{a_t} * (j in adapter-a_t block) and feed
    the masked inter into B_cat = [B[0,e]; ..; B[7,e]] ([128, 1024]).
    This turns the ragged adapter grouping into two dense matmuls and
    one elementwise mask — no on-device sorting or control flow.
  - The B-side matmul accumulates into the same PSUM tile as the base
    matmul, so base + lora is free.
  - All matmul operands are cast to bf16 on the host (fp32 PSUM
    accumulation on the PE); output is fp32.

The kernel is compiled for cap = max(group_sizes) rounded up to 128 and
cached per cap. All 8 cores run one SPMD program; per-core data differs
only through the input maps.
"""

import numpy as np

T, E, IN, OUT, A, R = 16384, 8, 1024, 1024, 8, 16
NCORES = 8
AR = A * R  # 128
KC = IN // 128  # 8 contraction chunks
OC = OUT // 512  # 2 output column chunks
WARMUP = 14  # PE warm-up matmuls. The HAM grants full clock ~4.8us after
# sustained PE activity starts; if the PE idles or bursts to full speed
# during the ramp it trips a 7-14us half-duty penalty window instead.
# Over-provisioning is near-free: post-grant warm-ups run at full clock.

_compiled_cache: dict[int, object] = {}


# ---------------------------------------------------------------------------
# walrus in this container accepts at most 1 sync-wait command per
# instruction; Tile attaches more. Split excess waits onto no-ops.
# ---------------------------------------------------------------------------


def _apply_tile_wait_patch():
    import bass_rust
    import concourse.tile as tile
    from concourse import mybir
    from concourse.vector_clock import ScopedClock

    if getattr(tile.TileContext, "_wait_split_patched", False):
        return

    MAX_WAITS = 1

    def _split_excess_waits(nc):
        for fn in nc.m.functions:
            for blk in fn.blocks:
                insts = blk.instructions  # live list
                i = 0
                while i < len(insts):
                    inst = insts[i]
                    si = inst.sync_info
                    if si is not None and len(si.on_wait) > MAX_WAITS:
                        waits = list(si.on_wait)
                        keep = waits[-MAX_WAITS:]
                        excess = waits[:-MAX_WAITS]
                        inst.sync_info = bass_rust.SyncInfo(
                            on_wait=keep, on_update=list(si.on_update)
                        )
                        pos = i
                        for k in range(0, len(excess), MAX_WAITS):
                            nop = mybir.InstNoOp(
                                name=f"{inst.name}-hoistw{k}",
                                engine=inst.engine,
                                bass_nofuse=True,
                                sync_info=mybir.SyncInfo(
                                    on_wait=excess[k : k + MAX_WAITS], on_update=[]
                                ),
                            )
                            insts.insert(pos, nop)
                            pos += 1
                            i += 1
                    i += 1

    def _split_drain_and_barrier(self, tick_clock, wait_clock):
        nc = self.nc
        drain_inst = nc.sync.drain()
        wait_clock.add_sem_waits(
            drain_inst.ins, ScopedClock({None: tick_clock.global_clock})
        )
        si = drain_inst.ins.sync_info
        if si is not None and len(si.on_wait) > MAX_WAITS:
            waits = list(si.on_wait)
            drain_inst.ins.sync_info = bass_rust.SyncInfo(
                on_wait=waits[:MAX_WAITS], on_update=list(si.on_update)
            )
            for k in range(MAX_WAITS, len(waits), MAX_WAITS):
                extra = nc.sync.drain()
                extra.ins.sync_info = bass_rust.SyncInfo(
                    on_wait=waits[k : k + MAX_WAITS], on_update=[]
                )

        import os as _os

        # Gather/release barrier: 2 sem hops (~1us) instead of the
        # 5-hop chained form (~2.5us).
        nc.all_engine_barrier(sem_only=True)
        assert self.sems is not None
        popped = nc._tile_sem_poison_stack.pop()
        assert popped is self._sem_poison
        nc.clear_and_free_semaphores(list(self.sems.allocated().values()))
        if _os.environ.get("LORA_LEAN_TAIL", "1") != "1":
            # Second barrier only matters for kernels that continue past
            # the TileContext; ours ends here (sem clears trail on gpsimd).
            nc.all_engine_barrier()

        _split_excess_waits(nc)

    tile.TileContext._drain_and_barrier = _split_drain_and_barrier
    tile.TileContext._wait_split_patched = True


# ---------------------------------------------------------------------------
# Bass program (one SPMD NeuronCore program, parameterized by cap)
# ---------------------------------------------------------------------------


def _build(cap: int):
    import concourse.bass as bass
    import concourse.tile as tile
    from concourse import mybir

    _apply_tile_wait_patch()

    ntt = cap // 128  # token tiles
    ngr = (cap + 511) // 512  # inter groups of up to 512 tokens

    bf16 = mybir.dt.bfloat16
    f32 = mybir.dt.float32
    f8e4 = mybir.dt.float8e4
    DR = mybir.MatmulPerfMode.DoubleRow

    # partition-id preamble (a ~1.1us TENSOR_LOAD on every engine) and
    # monotonic sems are unused here — skip them to shrink the preamble.
    # Also skip the single __init__ barrier (bass.py:7557): it only
    # orders the const-AP memsets, which nothing in this kernel reads.
    _orig_aeb = bass.Bass.all_engine_barrier
    bass.Bass.all_engine_barrier = lambda self, **kw: None
    try:
        nc = bass.Bass(enable_partition_id=False, monotonic_sem_count=0)
    finally:
        bass.Bass.all_engine_barrier = _orig_aeb
    # XT[g, k, p, c] = x_e[512g + c, 128k + p]
    XT = nc.dram_tensor("xt", [ngr, KC, 128, 512], bf16, kind="ExternalInput")
    XT8 = nc.dram_tensor("xt8", [ngr, KC, 128, 512], f8e4, kind="ExternalInput")
    W = nc.dram_tensor("w", [KC, 128, OUT], bf16, kind="ExternalInput")
    A8 = nc.dram_tensor("a8", [128, KC, AR], f8e4, kind="ExternalInput")
    BCAT = nc.dram_tensor("bcat", [AR, OUT], bf16, kind="ExternalInput")
    MASKT = nc.dram_tensor("maskt", [AR, cap], bf16, kind="ExternalInput")
    OUTD = nc.dram_tensor("out", [cap, OUT], bf16, kind="ExternalOutput")

    def gslice(g):
        t0 = g * 512
        return t0, min(512, cap - t0)

    with tile.TileContext(nc) as tc:
        with (
            tc.tile_pool(name="big", bufs=1) as big,
            tc.tile_pool(name="outp", bufs=2) as outp,
            tc.tile_pool(name="psi", bufs=2, space="PSUM") as psi,
            tc.tile_pool(name="pso", bufs=4, space="PSUM") as pso,
            tc.tile_pool(name="pswarm", bufs=1, space="PSUM") as pswarm,
        ):
            # DMA enqueue is ~600-850ns serial per dma_start, and all
            # in-flight transfers FAIR-SHARE the ~390 GB/s aggregate. So:
            # few big transfers, enqueued in priority order, with the bulk
            # x groups dep-chained behind W so they don't steal bandwidth
            # from the critical path.
            a8_sb = big.tile([128, KC, AR], f8e4)
            nc.sync.dma_start(a8_sb[:], A8[:])
            # Warm the PE/HAM through the input-DMA lead-in. Gating the
            # warm-up on the a8 DMA (~7.5us, uniform across cores) starts
            # sustained PE activity at a deterministic time: the HAM then
            # grants full clock ~4-5us later with no half-duty penalty.
            # Per-core engine-preamble jitter (the old vector-memset gate)
            # made slow cores trip the 7-14us half-duty window instead.
            wps = pswarm.tile([128, 512], f32)
            for i in range(WARMUP):
                nc.tensor.matmul(
                    wps[:], a8_sb[:, 0, :], a8_sb[:, i % 4 : i % 4 + 4, :],
                    start=(i == 0), stop=(i == WARMUP - 1),
                )
            # xt8_sb[p, g, k, c] = fp8(x_e[512g + c, 128k + p])
            xt8_sb = big.tile([128, ngr, KC, 512], f8e4)
            nc.sync.dma_start(
                xt8_sb[:, 0, :, :], XT8[0, :, :, :].rearrange("k p c -> p k c")
            )
            # xt_sb[p, g, k, c] = bf16 x for the base-matmul stationary
            xt_sb = big.tile([128, ngr, KC, 512], bf16)
            nc.sync.dma_start(
                xt_sb[:, 0, :, :], XT[0, :, :, :].rearrange("k p c -> p k c")
            )
            maskt_sb = big.tile([AR, cap], bf16)
            m0 = min(512, cap)
            nc.sync.dma_start(maskt_sb[:, :m0], MASKT[:, :m0])
            w_sb = big.tile([128, KC, OUT], bf16)
            for k in range(0, KC, 2):
                nc.sync.dma_start(
                    w_sb[:, k : k + 2, :],
                    W[k : k + 2, :, :].rearrange("k p c -> p k c"),
                )
            b_sb = big.tile([AR, OUT], bf16)
            nc.sync.dma_start(b_sb[:], BCAT[:])
            if cap > m0:
                nc.sync.dma_start(maskt_sb[:, m0:], MASKT[:, m0:])
            if ngr > 1:
                # Dummy op READING both the tail of W and the head of the
                # not-yet-loaded xt region: the bulk x DMAs then carry a
                # WAR dependency on it, so they wait for W to land before
                # competing for HBM bandwidth. (A write INTO xt instead
                # would race with the DMA and corrupt one element.)
                dep_sb = big.tile([128, 1], f32)
                nc.vector.scalar_tensor_tensor(
                    dep_sb[:],
                    w_sb[:, KC - 1, OUT - 1 : OUT],
                    1.0,
                    xt_sb[:, 1, 0, 0:1],
                    mybir.AluOpType.mult,
                    mybir.AluOpType.mult,
                )
                # Interleave fp8/bf16 x groups in consumption order:
                # phase1(g) runs after group g-1's last tile, tiles after.
                gmid = min(3, ngr)
                nc.sync.dma_start(
                    xt8_sb[:, 1:gmid, :, :],
                    XT8[1:gmid, :, :, :].rearrange("g k p c -> p g k c"),
                )
                nc.sync.dma_start(
                    xt_sb[:, 1:gmid, :, :],
                    XT[1:gmid, :, :, :].rearrange("g k p c -> p g k c"),
                )
                if ngr > gmid:
                    nc.sync.dma_start(
                        xt8_sb[:, gmid:, :, :],
                        XT8[gmid:, :, :, :].rearrange("g k p c -> p g k c"),
                    )
                    nc.sync.dma_start(
                        xt_sb[:, gmid:, :, :],
                        XT[gmid:, :, :, :].rearrange("g k p c -> p g k c"),
                    )

            interm_sb = big.tile([AR, cap], bf16)

            def phase1(g):
                # inter_all = (x8 @ (64*A_cat))^T via fp8 DoubleRow (two
                # 128-deep k-chunks per instruction), masked by s_a/64
                # -> interm_sb
                t0, wg = gslice(g)
                ps = psi.tile([128, 512], f32, name=f"psi{g}", tag="psi")
                for i in range(KC // 2):
                    nc.tensor.matmul(
                        ps[:, :wg],
                        a8_sb[:, 2 * i : 2 * i + 2, :],
                        xt8_sb[:, g, 2 * i : 2 * i + 2, :wg],
                        start=(i == 0),
                        stop=(i == KC // 2 - 1),
                        perf_mode=DR,
                    )
                nc.vector.scalar_tensor_tensor(
                    interm_sb[:, t0 : t0 + wg],
                    ps[:, :wg],
                    1.0,
                    maskt_sb[:, t0 : t0 + wg],
                    mybir.AluOpType.mult,
                    mybir.AluOpType.mult,
                )

            copy_engs = [nc.vector, nc.scalar]

            def dual_tile(tt0, o_sb, ncopy):
                # First two token tiles of group 0, fused k-major: one
                # W k-pair feeds 4 matmuls (2 tiles x 2 oc), matching the
                # ~1.35us/k-pair DMA delivery rate of W so the PE never
                # starves while W streams in.
                pss = [
                    pso.tile([128, 512], f32, name=f"psod{tt0 + t}_{i}", tag="pso")
                    for t in range(2)
                    for i in range(OC)
                ]
                for k in range(KC):
                    for t in range(2):
                        for oc in range(OC):
                            nc.tensor.matmul(
                                pss[t * OC + oc][:],
                                xt_sb[:, 0, k, (tt0 + t) * 128 : (tt0 + t) * 128 + 128],
                                w_sb[:, k, oc * 512 : oc * 512 + 512],
                                start=(k == 0),
                                stop=False,
                            )
                for t in range(2):
                    ts0 = (tt0 + t) * 128
                    for oc in range(OC):
                        nc.tensor.matmul(
                            pss[t * OC + oc][:],
                            interm_sb[:, ts0 : ts0 + 128],
                            b_sb[:, oc * 512 : oc * 512 + 512],
                            start=False,
                            stop=True,
                        )
                for t in range(2):
                    j = tt0 + t
                    for oc in range(OC):
                        eng = copy_engs[ncopy % 2]
                        ncopy += 1
                        dst = o_sb[:, j * OUT + oc * 512 : j * OUT + oc * 512 + 512]
                        if eng is nc.vector:
                            nc.vector.tensor_copy(dst, pss[t * OC + oc][:])
                        else:
                            nc.scalar.copy(dst, pss[t * OC + oc][:])
                return ncopy

            def token_tile(tt, o_sb, ncopy):
                # base k-loop into 2 psum banks (one per 512-wide output
                # chunk), + 1 lora matmul each, then copy to group stage.
                g, j = tt // 4, tt % 4
                ts0 = tt * 128
                pss = [
                    pso.tile([128, 512], f32, name=f"psod{tt}_{i}", tag="pso")
                    for i in range(OC)
                ]
                for k in range(KC):
                    for oc in range(OC):
                        nc.tensor.matmul(
                            pss[oc][:],
                            xt_sb[:, g, k, j * 128 : j * 128 + 128],
                            w_sb[:, k, oc * 512 : oc * 512 + 512],
                            start=(k == 0),
                            stop=False,
                        )
                for oc in range(OC):
                    nc.tensor.matmul(
                        pss[oc][:],
                        interm_sb[:, ts0 : ts0 + 128],
                        b_sb[:, oc * 512 : oc * 512 + 512],
                        start=False,
                        stop=True,
                    )
                for oc in range(OC):
                    eng = copy_engs[ncopy % 2]
                    ncopy += 1
                    dst = o_sb[:, j * OUT + oc * 512 : j * OUT + oc * 512 + 512]
                    if eng is nc.vector:
                        nc.vector.tensor_copy(dst, pss[oc][:])
                    else:
                        nc.scalar.copy(dst, pss[oc][:])
                return ncopy

            phase1(0)
            ncopy = 0
            for g in range(ngr):
                t0, wg = gslice(g)
                ntg = wg // 128
                # group output stage: o_sb[p, j*OUT + c] = out[t0+128j+p, c]
                o_sb = outp.tile(
                    [128, ntg * OUT], bf16, name=f"og{g}", tag="outp"
                )
                tts = list(range(t0 // 128, (t0 + wg) // 128))
                j = 0
                while j < len(tts):
                    if g == 0 and j == 0 and len(tts) >= 2:
                        ncopy = dual_tile(tts[0], o_sb, ncopy)
                        j += 2
                    else:
                        ncopy = token_tile(tts[j], o_sb, ncopy)
                        j += 1
                    # emit next group's phase 1 after this group's LAST
                    # token tile: by then its (dep-chained, late) x8 chunk
                    # has arrived, and the mask STT still completes before
                    # the first B-side matmul of group g+1 needs it.
                    if j == len(tts) and g + 1 < ngr:
                        phase1(g + 1)
                if g < ngr - 1:
                    # one output DMA per group: [128, ntg, OUT] rows
                    nc.sync.dma_start(
                        OUTD[t0 : t0 + wg, :].rearrange(
                            "(j p) c -> p j c", p=128
                        ),
                        o_sb[:].rearrange("p (j c) -> p j c", c=OUT),
                    )
                else:
                    # last group: per-(tile, oc) DMAs so the final
                    # transfer starts right after its own copy instead of
                    # waiting for all of the group's copies.
                    for j in range(wg // 128):
                        for oc in range(OC):
                            nc.sync.dma_start(
                                OUTD[
                                    t0 + j * 128 : t0 + j * 128 + 128,
                                    oc * 512 : oc * 512 + 512,
                                ],
                                o_sb[
                                    :,
                                    j * OUT + oc * 512 : j * OUT + oc * 512 + 512,
                                ],
                            )

    return nc


def _get_compiled(cap: int):
    if cap not in _compiled_cache:
        _compiled_cache[cap] = _build(cap)
    return _compiled_cache[cap]


# ---------------------------------------------------------------------------
# Host-side routing + execution
# ---------------------------------------------------------------------------


def _reference_numpy(x, group_sizes, adapter_indices_sorted, weight, lora_A, lora_B, lora_scaling):
    """Fallback replicating the jax reference exactly (only used for
    degenerate group_sizes that do not sum to T)."""
    x = np.asarray(x, np.float32)
    gs = np.asarray(group_sizes, np.int64)
    adapter = np.asarray(adapter_indices_sorted, np.int64)
    out = np.zeros((x.shape[0], weight.shape[2]), np.float32)
    # base: ragged_dot semantics (groups from cumsum, tail rows -> 0)
    offs = np.minimum(np.concatenate([[0], np.cumsum(gs)]), x.shape[0])
    for e in range(E):
        s, t = offs[e], offs[e + 1]
        if t > s:
            out[s:t] = x[s:t] @ weight[e]
    # lora: expert ids via repeat padded with the final value
    rep = np.repeat(np.arange(E), np.maximum(gs, 0))[: x.shape[0]]
    if rep.size == 0:
        rep = np.zeros(x.shape[0], np.int64)
    elif rep.size < x.shape[0]:
        rep = np.concatenate(
            [rep, np.full(x.shape[0] - rep.size, rep[-1], np.int64)]
        )
    for t in range(x.shape[0]):
        e, a = rep[t], adapter[t]
        inter = x[t] @ lora_A[a, e]
        out[t] += lora_scaling[a] * (inter @ lora_B[a, e])
    return out


def kernel(x, group_sizes, adapter_indices_sorted, weight, lora_A, lora_B, lora_scaling):
    import ml_dtypes

    x = np.ascontiguousarray(np.asarray(x, np.float32))
    weight = np.asarray(weight, np.float32)
    lora_A = np.asarray(lora_A, np.float32)
    lora_B = np.asarray(lora_B, np.float32)
    scaling = np.asarray(lora_scaling, np.float32)
    gs = np.asarray(group_sizes).astype(np.int64)
    adapter = np.asarray(adapter_indices_sorted).astype(np.int64)

    if gs.sum() != T or (gs < 0).any():
        return _reference_numpy(
            x, gs, adapter, weight, lora_A, lora_B, scaling
        )

    from concourse.bass_utils import run_bass_kernel_spmd

    bf = ml_dtypes.bfloat16
    f8 = ml_dtypes.float8_e4m3
    cap = int(max(128, -(-int(gs.max()) // 128) * 128))
    nc = _get_compiled(cap)

    offs = np.concatenate([[0], np.cumsum(gs)])
    in_maps = []
    for e in range(NCORES):
        n = int(gs[e])
        s = int(offs[e])
        ngr = (cap + 511) // 512
        xe = np.zeros((ngr * 512, IN), np.float32)
        xe[:n] = x[s : s + n]
        # [ngr, KC, 128, 512]: XT[g, k, p, t] = x_e[512g+t, 128k+p]
        xe_t = xe.T.reshape(KC, 128, ngr, 512).transpose(2, 0, 1, 3)
        xt = np.ascontiguousarray(xe_t.astype(bf))
        xt8 = np.ascontiguousarray(xe_t.astype(f8))
        w = np.ascontiguousarray(weight[e].reshape(KC, 128, OUT).astype(bf))
        # A_cat[:, a*R+r] = lora_A[a, e, :, r] -> [128, KC, AR], scaled by
        # 64 to land in fp8 e4m3's normal range; the mask divides it out.
        acat_full = lora_A[:, e].transpose(1, 0, 2).reshape(IN, AR)
        a8 = np.ascontiguousarray(
            (acat_full.reshape(KC, 128, AR) * 64.0).transpose(1, 0, 2).astype(f8)
        )
        bcat = np.ascontiguousarray(lora_B[:, e].reshape(AR, OUT).astype(bf))
        ae = adapter[s : s + n]
        m = np.zeros((A, cap), np.float32)
        m[ae, np.arange(n)] = scaling[ae] / 64.0
        maskt = np.ascontiguousarray(np.repeat(m, R, axis=0).astype(bf))
        in_maps.append(
            {"xt": xt, "xt8": xt8, "w": w, "a8": a8, "bcat": bcat, "maskt": maskt}
        )

    res = run_bass_kernel_spmd(nc, in_maps, list(range(NCORES)))

    out = np.empty((T, OUT), np.float32)
    for e in range(NCORES):
        n = int(gs[e])
        if n:
            out[int(offs[e]) : int(offs[e]) + n] = (
                res.results[e]["out"][:n].astype(np.float32)
            )
    return out

